# revision 1
# baseline (speedup 1.0000x reference)
"""Trainium2 Bass kernel for windowed sparse attention (nn_Attention_17703855194428).

Reference computation (per window w of 128 = B*X*Y, tokens N=294 = L*W1*W2):
    qkv = x_w @ w_qkv.T ; q,k,v heads (8 heads x 32 dim), q scaled by 1/sqrt(32)
    sim = q @ k.T + rel_pos_bias ; masked cols -> -1e9 ; softmax over keys
    out = (attn @ v) @ w_out.T

Sharding: pure data parallel over the 128 independent windows -> 16 windows
per NeuronCore, weights/bias replicated. No collectives.

Kernel layout strategy (per core, per window; all matmul inputs fp16,
fp32 PSUM accumulate; host pre-computes exp(rel-pos-bias) transposed and the
additive mask in the kernel's SBUF layouts):
    xT [C=256, N=294] (channels on partitions) ->
    q,k as [E, N] (head-dim on partitions), v as [N, E] (tokens on partitions)
    simT[j, i] = sum_d k[d,j] q[d,i] via row-tiled (K=32) matmuls, 2 heads
        per 2-bank psum tile, double-buffered (sim_pairs)
    P_T = exp(simT + mask_j) * exp(bias)_T   (one ACT exp with per-partition
        mask bias reading across psum banks + one DVE fp16 multiply)
    outU.T[hd, i] = sum_j v[j, hd] P_T[j, i] via col-tiled (M=32) matmuls
    rowsum broadcast to all 32 head rows for free via col-tiled ones-matmul
    1/rowsum via DVE reciprocal_approx_fast; normalize is one DVE multiply
    out = (outU.T * (1/rowsum)).T @ w_out.T via K=hd matmuls -> [tok, C],
    written fp16 (host upcasts to fp32)

Measured on HW (differential For_i timing): ~332 us per core for 16 windows
(1-stage software-pipelined emission: QKV of window w+1 ahead of attn of w).
PE is the bottleneck: this toolchain serializes every Ldweights+Matmult pair
(--enable-ldw-opt=false, per-MM sem updates), ~15.5 us/window of serial PE.
"""

import numpy as np
from contextlib import ExitStack

import concourse.bass as bass
import concourse.bacc as bacc
import concourse.mybir as mybir
from concourse import tile
from concourse.bass_utils import run_bass_kernel_spmd

import ml_dtypes

F32 = mybir.dt.float32
F32R = mybir.dt.float32r
BF16 = mybir.dt.bfloat16
FP16 = mybir.dt.float16
EXP = mybir.ActivationFunctionType.Exp

# Problem constants (hardcoded per harness contract)
B, AGENT, X, Y, WIN, DIM, HEADS, DH = 2, 6, 8, 8, 7, 256, 8, 32
N = AGENT * WIN * WIN            # 294 tokens per window
NWIN = B * X * Y                 # 128 windows
NCORES = 8
WPC = NWIN // NCORES             # 16 windows per core
JC = 98                          # key-chunk size (294 = 3*98)
NJC = 3
SCALE = DH ** -0.5
MASK_NEG = -1e9


def _rel_pos_index(L, Wh, Ww):
    coords = np.stack(np.meshgrid(np.arange(L), np.arange(Wh), np.arange(Ww), indexing="ij"))
    cf = coords.reshape(3, -1)
    rel = cf[:, :, None] - cf[:, None, :]
    rel = rel.transpose(1, 2, 0).astype(np.int64)
    rel[..., 0] += L - 1
    rel[..., 1] += Wh - 1
    rel[..., 2] += Ww - 1
    rel[..., 0] *= (2 * Wh - 1) * (2 * Ww - 1)
    rel[..., 1] *= 2 * Ww - 1
    return rel.sum(-1)  # (N, N) [i, j]


def build_graph(n_wins=WPC, wbufs=2, pvrs_bufs=2, misc_bufs=2, trace_sim=False, reps=1,
                v_eng='act', fo_eng='dve', gp_jc=(), sim_pairs=False, timing=False, out16=False, ab_noexp=False, ab_nosim=False, ab_nopv=False, pipe=False, sim_bufs=2, rs_presum=False):
    nc = bacc.Bacc(None)
    n_xt = 2 if timing else n_wins
    xt_d = nc.declare_dram_parameter("xt", [n_xt, 2, 128, N], FP16, isOutput=False)
    msk_d = nc.declare_dram_parameter("msk", [JC, n_wins * NJC], F32, isOutput=False)
    eb_d = nc.declare_dram_parameter("eb", [JC, HEADS, NJC, N], FP16, isOutput=False)
    wqkv_d = nc.declare_dram_parameter("wqkv", [2, 128, 3 * DIM], FP16, isOutput=False)
    wout_d = nc.declare_dram_parameter("wout", [2, 128, DIM], FP16, isOutput=False)
    ODT = FP16 if out16 else F32
    if timing:
        out_d = nc.dram_tensor("oscr", [n_wins, N, DIM], ODT)
        outx_d = nc.declare_dram_parameter("out", [1, N, DIM], ODT, isOutput=True)
    else:
        out_d = nc.declare_dram_parameter("out", [n_wins, N, DIM], ODT, isOutput=True)
        outx_d = None

    with tile.TileContext(nc, trace_sim=trace_sim) as tc, ExitStack() as ctx:
        cpool = ctx.enter_context(tc.tile_pool(name="consts", bufs=1))
        wpool = ctx.enter_context(tc.tile_pool(name="work", bufs=wbufs))
        # one xt slot per window: slot reuse on DMA-written tiles piles up
        # sync waits beyond what DMA descriptors support
        xpool = ctx.enter_context(tc.tile_pool(name="xin", bufs=n_wins))
        if sim_pairs:
            psim = ctx.enter_context(tc.tile_pool(name="psim", bufs=sim_bufs, space="PSUM"))
        else:
            psim = ctx.enter_context(tc.tile_pool(name="psim", bufs=1, space="PSUM"))
        pvrs = ctx.enter_context(tc.tile_pool(name="pvrs", bufs=pvrs_bufs, space="PSUM"))
        ps1 = ctx.enter_context(tc.tile_pool(name="ps1", bufs=misc_bufs, space="PSUM"))

        # ---- replicated constants ----
        wqkv_sb = []
        for c in range(2):
            t = cpool.tile([128, 3 * DIM], FP16, tag=f"wqkv{c}")
            nc.sync.dma_start(t[:], wqkv_d[c])
            wqkv_sb.append(t)
        wout_sb = []
        for c in range(2):
            t = cpool.tile([128, DIM], FP16, tag=f"wout{c}")
            nc.sync.dma_start(t[:], wout_d[c])
            wout_sb.append(t)
        eb_sb = cpool.tile([JC, HEADS, NJC, N], FP16, tag="eb")
        nc.sync.dma_start(eb_sb[:], eb_d[:])
        msk_sb = cpool.tile([JC, n_wins * NJC], F32, tag="msk")
        nc.sync.dma_start(msk_sb[:], msk_d[:])
        ones_sb = cpool.tile([JC, 32], FP16, tag="ones")
        nc.vector.memset(ones_sb[:], 1.0)

        if ab_noexp or ab_nosim:
            pt_const = cpool.tile([JC, 4, N], FP16, tag="ptc")
            nc.vector.memset(pt_const[:], 0.01)
            stub = cpool.tile([1, 8], F32, tag="stub")

        # warm-up touches: absorb the one-time const-DMA waits into throwaway
        # instructions so steady-state ops stay within the per-instruction
        # sync-wait budget
        scr_a = cpool.tile([JC, 1], F32, tag="scr_a")
        nc.scalar.copy(scr_a[:], msk_sb[:, 0:1])
        scr_v = cpool.tile([JC, 1], FP16, tag="scr_v")
        nc.vector.tensor_copy(scr_v[:], eb_sb[:, 0, 0, 0:1])

        def emit_qkv(w):
            xt_t = []
            for c in range(2):
                t = xpool.tile([128, N], FP16, tag=f"xt{c}")
                nc.sync.dma_start(t[:], xt_d[w % n_xt, c])
                xt_t.append(t)
            qk_sb = []
            for p in range(4):
                ps = ps1.tile([128, 512], F32, tag="b1")
                for c in range(2):
                    nc.tensor.matmul(
                        ps[:, 0:N],
                        lhsT=wqkv_sb[c][:, 128 * p:128 * (p + 1)],
                        rhs=xt_t[c][:],
                        start=(c == 0), stop=(c == 1),
                    )
                t = wpool.tile([128, N], FP16, tag=f"qk{p}")
                if p < 2:
                    nc.scalar.copy(t[:], ps[:, 0:N])
                else:
                    nc.vector.tensor_copy(t[:], ps[:, 0:N])
                qk_sb.append(t)
            v_sb = []
            for j in range(NJC):
                ps = ps1.tile([128, 512], F32, tag="b1")
                for c in range(2):
                    nc.tensor.matmul(
                        ps[0:JC, 0:DIM],
                        lhsT=xt_t[c][:, JC * j:JC * (j + 1)],
                        rhs=wqkv_sb[c][:, 2 * DIM:3 * DIM],
                        start=(c == 0), stop=(c == 1),
                    )
                t = wpool.tile([JC, DIM], FP16, tag=f"v{j}")
                if v_eng == 'act':
                    nc.scalar.copy(t[:], ps[0:JC, 0:DIM])
                else:
                    nc.vector.tensor_copy(t[:], ps[0:JC, 0:DIM])
                v_sb.append(t)
            return qk_sb, v_sb

        def emit_attn(w, qk_sb, v_sb):
            on_sb = []
            for hg in range(2):
                pv = pvrs.tile([128, 512], F32, tag="pvrs")
                rs = pvrs.tile([128, 512], F32, tag="pvrs")
                pts = []
                if ab_nosim:
                    pts = [pt_const] * NJC
                elif sim_pairs:
                    for jc in range(NJC):
                        ptj = wpool.tile([JC, 4, N], FP16, tag=f"pt{jc}")
                        for sg in range(2):
                            smp = psim.tile([128, 1024], F32, tag="sim")
                            for i2 in range(2):
                                t4 = 2 * sg + i2
                                nc.tensor.matmul(
                                    smp[0:JC, 512 * i2:512 * i2 + N],
                                    lhsT=qk_sb[2 + hg][32 * t4:32 * (t4 + 1), JC * jc:JC * (jc + 1)],
                                    rhs=qk_sb[hg][32 * t4:32 * (t4 + 1), :],
                                    start=True, stop=True,
                                    tile_position=(32 * t4, 0),
                                )
                            et = wpool.tile([JC, 2, N], FP16, tag=f"et{jc}{sg}")
                            sim_ap = smp[0:JC, :].rearrange("p (t x) -> p t x", t=2)[:, :, 0:N]
                            nc.scalar.activation(
                                et[:], sim_ap, EXP,
                                bias=msk_sb[:, NJC * w + jc:NJC * w + jc + 1],
                            )
                            eb_ap = eb_sb[:, 4 * hg + 2 * sg:4 * hg + 2 * sg + 2, jc, :]
                            nc.vector.tensor_mul(ptj[:, 2 * sg:2 * sg + 2, :], et[:], eb_ap)
                        pts.append(ptj)
                else:
                    for jc in range(NJC):
                        smp = psim.tile([128, 2048], F32, tag="sim")
                        for t4 in range(4):
                            nc.tensor.matmul(
                                smp[0:JC, 512 * t4:512 * t4 + N],
                                lhsT=qk_sb[2 + hg][32 * t4:32 * (t4 + 1), JC * jc:JC * (jc + 1)],
                                rhs=qk_sb[hg][32 * t4:32 * (t4 + 1), :],
                                start=True, stop=True,
                                tile_position=(32 * t4, 0),
                            )
                        if ab_noexp:
                            nc.scalar.copy(stub[:], smp[0:1, 0:8])
                            pts.append(pt_const)
                            continue
                        et = wpool.tile([JC, 4, N], FP16, tag=f"et{jc}")
                        sim_ap = smp[0:JC, :].rearrange("p (t x) -> p t x", t=4)[:, :, 0:N]
                        nc.scalar.activation(
                            et[:], sim_ap, EXP,
                            bias=msk_sb[:, NJC * w + jc:NJC * w + jc + 1],
                        )
                        pt = wpool.tile([JC, 4, N], FP16, tag=f"pt{jc}")
                        eb_ap = eb_sb[:, 4 * hg:4 * (hg + 1), jc, :]
                        nc.vector.tensor_mul(pt[:], et[:], eb_ap)
                        pts.append(pt)

                ptsum = None
                if rs_presum and not (ab_nopv or ab_nosim or ab_noexp):
                    # rowsum needs sum over all 294 j; sum the three j-chunks
                    # on DVE first so each head needs one ones-matmul, not 3
                    ptsum = wpool.tile([JC, 4, N], FP16, tag="ptsum")
                    nc.vector.tensor_add(ptsum[:], pts[0][:], pts[1][:])
                    nc.vector.tensor_add(ptsum[:], ptsum[:], pts[2][:])
                pv_iters = [(0, [0])] if ab_nopv else [(t, list(range(NJC))) for t in range(4)]
                for t4, jcs in pv_iters:
                    h = 4 * hg + t4
                    for jc in jcs:
                        nc.tensor.matmul(
                            pv[32 * t4:32 * (t4 + 1), 0:N],
                            lhsT=v_sb[jc][:, 32 * h:32 * (h + 1)],
                            rhs=pts[jc][:, t4, :],
                            start=(jc == 0), stop=(jc == jcs[-1]),
                            tile_position=(0, 32 * t4),
                            skip_group_check=True,
                        )
                    if ptsum is not None:
                        nc.tensor.matmul(
                            rs[32 * t4:32 * (t4 + 1), 0:N],
                            lhsT=ones_sb[:],
                            rhs=ptsum[:, t4, :],
                            start=True, stop=True,
                            tile_position=(0, 32 * t4),
                            skip_group_check=True,
                        )
                        continue
                    for jc in jcs:
                        nc.tensor.matmul(
                            rs[32 * t4:32 * (t4 + 1), 0:N],
                            lhsT=ones_sb[:],
                            rhs=pts[jc][:, t4, :],
                            start=(jc == 0), stop=(jc == jcs[-1]),
                            tile_position=(0, 32 * t4),
                            skip_group_check=True,
                        )
                rr = wpool.tile([128, N], F32, tag="rr")
                nc.vector.reciprocal_approx_fast(rr[:], rs[:, 0:N])
                on = wpool.tile([128, N], FP16, tag=f"on{hg}")
                nc.vector.tensor_mul(on[:], pv[:, 0:N], rr[:])
                on_sb.append(on)

            for ic in range(NJC):
                po = ps1.tile([128, 512], F32, tag="b1")
                for kc in range(2):
                    nc.tensor.matmul(
                        po[0:JC, 0:DIM],
                        lhsT=on_sb[kc][:, JC * ic:JC * (ic + 1)],
                        rhs=wout_sb[kc][:],
                        start=(kc == 0), stop=(kc == 1),
                    )
                fo = wpool.tile([JC, DIM], FP16 if out16 else F32, tag=f"fo{ic}")
                if fo_eng == 'dve':
                    nc.vector.tensor_copy(fo[:], po[0:JC, 0:DIM])
                else:
                    nc.scalar.copy(fo[:], po[0:JC, 0:DIM])
                nc.sync.dma_start(out_d[w, JC * ic:JC * (ic + 1), :], fo[:])
                if timing and w == 0:
                    nc.sync.dma_start(outx_d[0, JC * ic:JC * (ic + 1), :], fo[:])

        rep_ctx = tc.For_i(0, reps, 1) if reps > 1 else None
        if rep_ctx is not None:
            ctx.enter_context(rep_ctx)
        if pipe:
            # 1-stage software pipeline: QKV of window w+1 is emitted (and
            # thus prioritized) ahead of attention of window w, so the PE
            # fills softmax-chain stalls with the next window's projections
            prev = None
            for w in range(n_wins):
                cur = emit_qkv(w)
                if prev is not None:
                    emit_attn(w - 1, *prev)
                prev = cur
            emit_attn(n_wins - 1, *prev)
        else:
            for w in range(n_wins):
                qk_sb, v_sb = emit_qkv(w)
                emit_attn(w, qk_sb, v_sb)

    nc.compile()
    return nc


def host_prep(x, mask, w_qkv, w_out, bias_table):
    """Build per-core input maps (numpy only)."""
    x = np.asarray(x, dtype=np.float32)
    mask = np.asarray(mask)
    w_qkv = np.asarray(w_qkv, dtype=np.float32)
    w_out = np.asarray(w_out, dtype=np.float32)
    bias_table = np.asarray(bias_table, dtype=np.float32)

    # x: (B, L, X, Y, W1, W2, C) -> windows (B,X,Y) x [C, N]
    xr = np.ascontiguousarray(x.transpose(0, 2, 3, 1, 4, 5, 6)).reshape(NWIN, N, DIM)
    xt = np.ascontiguousarray(xr.transpose(0, 2, 1)).reshape(NWIN, 2, 128, N).astype(np.float16)

    # mask: (B, X, Y, W1, W2, 1, L) -> (B,X,Y) x N with token order (l, w1, w2)
    m = np.ascontiguousarray(mask.transpose(0, 1, 2, 5, 6, 3, 4)).reshape(NWIN, N)
    maskadd = np.where(m == 0, np.float32(MASK_NEG), np.float32(0.0)).astype(np.float32)

    # exp(bias) transposed: ebT[h, j, i] = exp(bias[i, j, h])
    ri = _rel_pos_index(AGENT, WIN, WIN)
    bias = bias_table[ri]                       # (N, N, H) [i, j, h]
    ebT = np.exp(bias.transpose(2, 1, 0))       # (H, j, i)
    eb_host = np.ascontiguousarray(
        ebT.reshape(HEADS, NJC, JC, N).transpose(2, 0, 1, 3)
    ).astype(np.float16)                # (JC, H, NJC, N)

    wq = w_qkv.copy()
    wq[0:DIM] *= np.float32(SCALE)
    wqkvT = np.ascontiguousarray(wq.T).reshape(2, 128, 3 * DIM).astype(np.float16)
    woutT = np.ascontiguousarray(w_out.T).reshape(2, 128, DIM).astype(np.float16)

    in_maps = []
    for core in range(NCORES):
        ws = slice(WPC * core, WPC * (core + 1))
        mm = maskadd[ws].reshape(WPC, NJC, JC).transpose(2, 0, 1).reshape(JC, WPC * NJC)
        in_maps.append({
            "xt": np.ascontiguousarray(xt[ws]),
            "msk": np.ascontiguousarray(mm),
            "eb": eb_host,
            "wqkv": wqkvT,
            "wout": woutT,
        })
    return in_maps


def assemble_output(core_outs):
    """core_outs: list of [WPC, N, DIM] arrays -> full (B, L, X, Y, W1, W2, C)."""
    out = np.concatenate([np.asarray(o) for o in core_outs], axis=0).astype(np.float32)
    out = out.reshape(B, X, Y, AGENT, WIN, WIN, DIM)
    return np.ascontiguousarray(out.transpose(0, 3, 1, 2, 4, 5, 6)).astype(np.float32)


_NC_CACHE = {}


def _get_nc(n_wins=WPC):
    if n_wins not in _NC_CACHE:
        _NC_CACHE[n_wins] = build_graph(n_wins, **BEST_CFG)
    return _NC_CACHE[n_wins]


BEST_CFG = dict(wbufs=4, v_eng="dve", out16=True, sim_pairs=True, fo_eng="act", pipe=True)


def kernel(x, mask, w_qkv, w_out, bias_table):
    in_maps = host_prep(x, mask, w_qkv, w_out, bias_table)
    nc = _get_nc(WPC)
    res = run_bass_kernel_spmd(nc, in_maps, core_ids=list(range(NCORES)))
    core_outs = [res.results[i]["out"] for i in range(NCORES)]
    return assemble_output(core_outs)

